# revision 52
# baseline (speedup 1.0000x reference)
"""Trainium2 Bass kernel for nn_DenseContrastLoss (v3).

Strategy (data-parallel over instances, 8 cores, 13 instances each):
  - Host: the two 1x1-conv projections are per-pixel, so only the 96
    sampled pixels per instance ever matter.  Gather them on the host
    (trivial numpy fancy-indexing) and ship a dense channel-major
    [256 x 1248] bf16 matrix per core -- no GPSIMD gathers, no on-device
    transposes, ~0.65 MB of input instead of 10.4 MB.
  - Device: 2-layer projection head as bf16 matmuls (PSUM f32 accum),
    L2 normalization via square/ones-matmul-colsum/exp(-0.5 ln(tau x)),
    then per-instance 32x64 [sim_ap | sim_an] similarity blocks packed
    4 instances per 128 PSUM partitions (PE column tiling).  The InfoNCE
    loss skips the max-subtraction (it cancels exactly; |logit| <= 1/tau
    so exp stays finite) and runs per column-group so only the last
    group's chain sits past the final sims.  A mask matmul reduces over
    the 32 anchors; host divides by 32*32, applies the validity mask and
    the loss weight.
  - Column chunks are [512, 512, 224] so the last chunk's serial
    square->colsum->ln->exp->normalize chain is short.
"""

import os
import sys

import numpy as np

if "/opt/trn_rl_repo" not in sys.path:
    sys.path.insert(0, "/opt/trn_rl_repo")

import concourse.bass as bass
import concourse.tile as tile
from concourse import bacc, mybir
from concourse.bass_utils import run_bass_kernel_spmd

try:
    from ml_dtypes import bfloat16 as np_bf16
except ImportError:  # ml_dtypes ships with jax
    import jax.numpy as _jnp

    np_bf16 = _jnp.bfloat16

F32 = mybir.dt.float32
BF16 = mybir.dt.bfloat16

TAU = 0.07
LOSS_WEIGHT = 1.2
NUM_SAMPLES = 32
C = 256
SIDE = 28
PIX = SIDE * SIDE
N_INST = 100
N_CORES = 8
NI = 13                      # instances per core (8*13 = 104 >= 100)
SAMP = 3 * NUM_SAMPLES       # 96 sampled pixels per instance
S = NI * SAMP                # 1248 columns per core
GH = S // 2                  # 624-column DMA halves
CHS = [(0, 512), (512, 512), (1024, 224)]
# contiguous DMA pieces of the flat gtd layout: chunks 0 and 1 are split
# in two 256-col halves so both HWDGE queues fetch them in parallel
PIECES = [(0, 256), (256, 256), (512, 256), (768, 256), (1024, 224)]
NWARM = int(os.environ.get("NWARM", "7"))


def _build_nc():
    nc = bacc.Bacc("TRN2", target_bir_lowering=False)
    gtd = nc.declare_dram_parameter("gtd", [128, 2 * S], BF16, isOutput=False)
    w1d = nc.declare_dram_parameter("w1d", [128, 2, 2, 128], BF16, isOutput=False)
    w2d = nc.declare_dram_parameter("w2d", [128, 2, 2, 128], BF16, isOutput=False)
    auxd = nc.declare_dram_parameter("auxd", [128, 4], F32, isOutput=False)
    lossd = nc.declare_dram_parameter("loss", [128, 4], F32, isOutput=True)

    AT = mybir.ActivationFunctionType
    ALU = mybir.AluOpType
    PSUM = bass.MemorySpace.PSUM
    X = mybir.AxisListType.X

    with tile.TileContext(nc) as tc:
        with tc.tile_pool(name="singles", bufs=1) as sg:
            W1 = sg.tile([128, 2, 2, 128], BF16)
            W2 = sg.tile([128, 2, 2, 128], BF16)
            AUX = sg.tile([128, 4], F32)
            gtc = [
                sg.tile([128, 2, w], BF16, name=f"gt{c}")
                for c, (c0, w) in enumerate(CHS)
            ]
            ones128 = sg.tile([128, 128], BF16)
            wmov = sg.tile([128, 512], BF16)

            nc.vector.memset(ones128[:], 1.0)
            nc.vector.memset(wmov[:], 1.0)

            # One ACT table set covers every function used here (relu,
            # square, ln, exp, copy).  Preload it first on the scalar
            # engine so the framework never auto-inserts a second load.
            nc.scalar.add_instruction(
                mybir.InstLoadActFuncSet(
                    name=nc.get_next_instruction_name(),
                    ins=[],
                    outs=[],
                    act_func_set_id=6,  # natural_log_exp_and_others
                )
            )

            # Input DMAs split across the two HWDGE queues (sync + scalar),
            # pieces aligned to compute chunks, ordered by first use.
            # piece i covers flat elements [2*o, 2*(o+w)) = chunk cols with
            # both k-halves; SBUF side strides into the right gtc tile.
            def gpiece(eng, i):
                o, w = PIECES[i]
                c = 0 if o < 512 else (1 if o < 1024 else 2)
                cbase = CHS[c][0]
                eng.dma_start(
                    out=gtc[c][:, :, o - cbase : o - cbase + w],
                    in_=gtd[:, 2 * o : 2 * (o + w)].rearrange(
                        "p (k j) -> p k j", k=2
                    ),
                )

            gpiece(nc.sync, 0)
            gpiece(nc.scalar, 1)
            nc.sync.dma_start(out=W1[:], in_=w1d[:, :, :, :])
            nc.scalar.dma_start(out=AUX[:], in_=auxd[:, :])
            gpiece(nc.sync, 2)
            gpiece(nc.scalar, 3)
            nc.scalar.dma_start(out=W2[:], in_=w2d[:, :, :, :])
            gpiece(nc.sync, 4)

            BIA = AUX[:, 0:4]

            # PE warm-up during the DMA window: full-width streams so the
            # tensor-engine clock actually ramps before real data lands.
            with tc.tile_pool(name="warmp", bufs=2, space=PSUM) as warmp:
                for _ in range(NWARM):
                    wt = warmp.tile([128, 512], F32, tag="warm")
                    nc.tensor.matmul(
                        wt[:], ones128[:], wmov[:], start=True, stop=True
                    )

            with (
                tc.tile_pool(name="big", bufs=1) as big,
                tc.tile_pool(name="l1p", bufs=3, space=PSUM) as l1p,
                tc.tile_pool(name="l2p", bufs=2, space=PSUM) as l2p,
                tc.tile_pool(name="nsqp", bufs=2, space=PSUM) as nsqp,
                tc.tile_pool(name="simp", bufs=1, space=PSUM) as simp,
            ):
                HS = big.tile([128, 2, S], BF16, name="HS")
                PS = big.tile([128, 2, S], BF16, name="PS")
                QS = big.tile([128, 2, S], BF16, name="QS")
                PN = big.tile([128, 2, S], BF16, name="PN")
                RR = big.tile([128, S], BF16, name="RR")
                spb = simp.tile([128, 256], F32, tag="sp")
                sp = spb[:, 0:256]
                pqs = {}

                def csl(c):
                    c0, w = CHS[c]
                    return slice(c0, c0 + w)

                def l1(c, m, eng):
                    c0, w = CHS[c]
                    pp = l1p.tile([128, 512], F32, tag="pp")
                    for k in (0, 1):
                        nc.tensor.matmul(
                            pp[:, :w], W1[:, m, k], gtc[c][:, k],
                            start=(k == 0), stop=(k == 1),
                        )
                    if eng == "v":
                        nc.vector.tensor_scalar(
                            out=HS[:, m, csl(c)], in0=pp[:, :w],
                            scalar1=BIA[:, m : m + 1], scalar2=0.0,
                            op0=ALU.add, op1=ALU.max,
                        )
                    else:
                        nc.scalar.activation(
                            out=HS[:, m, csl(c)], in_=pp[:, :w], func=AT.Relu,
                            bias=BIA[:, m : m + 1],
                        )

                def l2mm(c, m):
                    c0, w = CHS[c]
                    pq = l2p.tile([128, 512], F32, tag="pq")
                    pqs[(c, m)] = pq
                    for k in (0, 1):
                        nc.tensor.matmul(
                            pq[:, :w], W2[:, m, k], HS[:, k, csl(c)],
                            start=(k == 0), stop=(k == 1),
                        )

                def ps_op(c, m):
                    c0, w = CHS[c]
                    nc.vector.tensor_scalar_add(
                        out=PS[:, m, csl(c)], in0=pqs[(c, m)][:, :w],
                        scalar1=BIA[:, 2 + m : 3 + m],
                    )

                def qs_act(c, m):
                    c0, w = CHS[c]
                    nc.scalar.activation(
                        out=QS[:, m, csl(c)], in_=pqs[(c, m)][:, :w],
                        func=AT.Square, bias=BIA[:, 2 + m : 3 + m],
                    )

                def qs_dve(c, m):
                    nc.vector.tensor_mul(
                        out=QS[:, m, csl(c)], in0=PS[:, m, csl(c)],
                        in1=PS[:, m, csl(c)],
                    )

                def nsqmm(c):
                    c0, w = CHS[c]
                    nq = nsqp.tile([128, 512], F32, tag="nq")
                    for m in (0, 1):
                        nc.tensor.matmul(
                            nq[:, :w], ones128[:], QS[:, m, csl(c)],
                            start=(m == 0), stop=(m == 1),
                        )
                    return nq

                def ln_op(c, nq):
                    c0, w = CHS[c]
                    lnt = big.tile([128, 512], F32, tag="lnt", bufs=2)
                    nc.scalar.activation(
                        out=lnt[:, :w], in_=nq[:, :w], func=AT.Ln, scale=float(TAU)
                    )
                    return lnt

                def exp_op(c, lnt):
                    c0, w = CHS[c]
                    nc.scalar.activation(
                        out=RR[:, csl(c)], in_=lnt[:, :w], func=AT.Exp, scale=-0.5
                    )

                def pn_op(c):
                    c0, w = CHS[c]
                    nc.vector.tensor_mul(
                        out=PN[:, :, csl(c)], in0=PS[:, :, csl(c)],
                        in1=RR[:, csl(c)].unsqueeze(1).broadcast_to([128, 2, w]),
                    )

                def sims(g):
                    for q in range(4):
                        n = 4 * g + q
                        if n >= NI:
                            continue
                        a0 = SAMP * n
                        for k in (0, 1):
                            nc.tensor.matmul(
                                sp[32 * q : 32 * q + 32, 64 * g : 64 * g + 64],
                                PN[:, k, a0 : a0 + 32],
                                PN[:, k, a0 + 32 : a0 + 96],
                                start=(k == 0), stop=(k == 1),
                                tile_position=(0, 32 * q),
                            )

                # pad slots (g=3, q>=1) are never written by a matmul;
                # partition-offset accesses may span at most 32 partitions
                for q in (1, 2, 3):
                    nc.vector.memset(sp[32 * q : 32 * q + 32, 192:256], 0.0)

                # ---- loss tail, two halves of [128, 128] ----
                # loss_km = ln(exp(s_ap) + sum_j exp(s_an_j)) - s_ap
                ee = big.tile([128, 256], F32, name="ee")
                ssum = big.tile([128, 4], F32, name="ssum")
                tt = big.tile([128, 128], F32, name="tt")
                lg = big.tile([128, 128], F32, name="lg")
                ctb = big.tile([128, 128], F32, name="ctb")
                rowr = big.tile([128, 4], F32, name="rowr")

                def tail_exp(h):
                    nc.scalar.activation(
                        out=ee[:, 128 * h : 128 * h + 128],
                        in_=sp[:, 128 * h : 128 * h + 128], func=AT.Exp,
                    )

                def tail_dve1(h):
                    eev = ee[:, 128 * h : 128 * h + 128].rearrange(
                        "p (g x) -> p g x", g=2
                    )
                    nc.vector.reduce_sum(
                        out=ssum[:, 2 * h : 2 * h + 2], in_=eev[:, :, 32:64], axis=X
                    )
                    nc.vector.tensor_add(
                        out=tt[:, 64 * h : 64 * h + 64].rearrange(
                            "p (g x) -> p g x", g=2
                        ),
                        in0=eev[:, :, 0:32],
                        in1=ssum[:, 2 * h : 2 * h + 2]
                        .unsqueeze(-1)
                        .broadcast_to([128, 2, 32]),
                    )

                def tail_ln(h):
                    nc.scalar.activation(
                        out=lg[:, 64 * h : 64 * h + 64],
                        in_=tt[:, 64 * h : 64 * h + 64], func=AT.Ln,
                    )

                def tail_dve2(h):
                    # fused (lg - s_ap) with per-partition row-sum accumulate
                    for g in (2 * h, 2 * h + 1):
                        nc.vector.scalar_tensor_tensor(
                            out=ctb[:, 32 * g : 32 * g + 32],
                            in0=lg[:, 32 * g : 32 * g + 32],
                            scalar=0.0,
                            in1=sp[:, 64 * g : 64 * g + 32],
                            op0=ALU.add,
                            op1=ALU.subtract,
                            accum_out=rowr[:, g : g + 1],
                        )

                # ---- emission (per-engine program order matters) ----
                # PE: warm, L1(0), L1(1), L2(0), L1(2), L2(1), L2(2),
                #     nsq(0..2), sims(0..3), lp  -- all L2 before any nsq
                #     so nsq's qs-waits never idle the PE.
                l1(0, 0, "v"); l1(0, 1, "v")
                l1(1, 0, "s"); l1(1, 1, "s")
                l2mm(0, 0); l2mm(0, 1)
                qs_act(0, 0); ps_op(0, 0); ps_op(0, 1); qs_dve(0, 1)
                l1(2, 0, "v"); l1(2, 1, "s")
                l2mm(1, 0); l2mm(1, 1)
                qs_act(1, 0); ps_op(1, 0); ps_op(1, 1); qs_dve(1, 1)
                nq0 = nsqmm(0)
                ln0 = ln_op(0, nq0)
                exp_op(0, ln0)
                l2mm(2, 0); l2mm(2, 1)
                nq1 = nsqmm(1)
                qs_act(2, 0); ps_op(2, 1); qs_dve(2, 1); ps_op(2, 0)
                ln1 = ln_op(1, nq1)
                exp_op(1, ln1)
                nq2 = nsqmm(2)
                ln2 = ln_op(2, nq2)
                exp_op(2, ln2)
                pn_op(0); pn_op(1); pn_op(2)
                sims(0); sims(1); sims(2); sims(3)
                # stage-major tail emission: each engine's queue never has a
                # later-ready op ahead of an earlier-ready one
                tail_exp(0); tail_exp(1)
                tail_dve1(0); tail_dve1(1)
                tail_ln(0); tail_ln(1)
                tail_dve2(0); tail_dve2(1)

                # host sums each 32-anchor partition block of rowr
                nc.sync.dma_start(out=lossd[:, :], in_=rowr[:])

    nc.compile()
    return nc


_NC_CACHE = None


def _get_nc():
    global _NC_CACHE
    if _NC_CACHE is None:
        _NC_CACHE = _build_nc()
    return _NC_CACHE


def _host_prep(feats, w1, b1, w2, b2, anchor_inds, pos_inds, neg_inds):
    """Build the 8 per-core input maps (host-side gather + packing)."""
    ff = np.asarray(feats, np.float32).reshape(N_INST, C, PIX)

    def flat(i):
        i = np.asarray(i)
        return i[..., 0].astype(np.int64) * SIDE + i[..., 1].astype(np.int64)

    idx = np.concatenate(
        [flat(anchor_inds), flat(pos_inds), flat(neg_inds)], axis=1
    )  # [100, 96]
    ntot = N_CORES * NI
    inst = np.arange(ntot) % N_INST  # wrap the 4 pad rows

    G = np.take_along_axis(ff[inst], idx[inst][:, None, :], axis=2)  # [104,256,96]
    G = G.reshape(N_CORES, NI, C, SAMP).transpose(0, 2, 1, 3)  # [8,256,13,96]
    G = G.reshape(N_CORES, 2, 128, S)  # [core, k, p, col]
    # piece-major, per-piece [k, col] contiguous per partition row
    pieces = [
        G[:, :, :, o : o + w].transpose(0, 2, 1, 3).reshape(N_CORES, 128, 2 * w)
        for (o, w) in PIECES
    ]
    gtd = np.ascontiguousarray(np.concatenate(pieces, axis=2)).astype(np_bf16)

    def wpack(w):
        wa = np.asarray(w, np.float32).reshape(2, 128, 2, 128)  # [m,i,k,p]
        # device layout [p, m, k, i]
        return np.ascontiguousarray(wa.transpose(3, 0, 2, 1)).astype(np_bf16)

    w1d = wpack(w1)
    w2d = wpack(w2)
    b1r = np.asarray(b1, np.float32).reshape(2, 128).T  # [128, 2]
    b2r = np.asarray(b2, np.float32).reshape(2, 128).T
    auxd = np.ascontiguousarray(np.concatenate([b1r, b2r], axis=1))  # [128, 4]

    return [
        {"gtd": gtd[c], "w1d": w1d, "w2d": w2d, "auxd": auxd}
        for c in range(N_CORES)
    ]


def _finalize(loss_per, gt_mask):
    gt = np.asarray(gt_mask)
    area = gt.reshape(gt.shape[0], -1).sum(axis=1)
    valid = (area > NUM_SAMPLES) & (area < PIX - NUM_SAMPLES)
    n_valid = np.float32(valid.sum())
    if n_valid > 0:
        total = np.float32(np.where(valid, loss_per, 0.0).astype(np.float32).sum())
        out = total / max(n_valid, np.float32(1.0))
    else:
        out = np.float32(0.0)
    return np.float32(out * np.float32(LOSS_WEIGHT))


def kernel(feats, w1, b1, w2, b2, gt_mask, anchor_inds, pos_inds, neg_inds,
           _results_hook=None):
    nc = _get_nc()
    in_maps = _host_prep(feats, w1, b1, w2, b2, anchor_inds, pos_inds, neg_inds)
    res = run_bass_kernel_spmd(nc, in_maps, list(range(N_CORES)))
    if _results_hook is not None:
        _results_hook(res)
    loss_per = np.zeros(N_CORES * NI, np.float32)
    for c in range(N_CORES):
        lo = np.asarray(res.results[c]["loss"], np.float32)  # [128, 4]
        blk = lo.reshape(4, 32, 4).sum(axis=1)  # [q, g]
        for n in range(NI):
            loss_per[NI * c + n] = blk[n % 4, n // 4]
    loss_per = loss_per[:N_INST] / float(NUM_SAMPLES * NUM_SAMPLES)
    return _finalize(loss_per, gt_mask)


# revision 53
# speedup vs baseline: 1.0392x; 1.0392x over previous
"""Trainium2 Bass kernel for nn_DenseContrastLoss (v3).

Strategy (data-parallel over instances, 8 cores, 13 instances each):
  - Host: the two 1x1-conv projections are per-pixel, so only the 96
    sampled pixels per instance ever matter.  Gather them on the host
    (trivial numpy fancy-indexing) and ship a dense channel-major
    [256 x 1248] bf16 matrix per core -- no GPSIMD gathers, no on-device
    transposes, ~0.65 MB of input instead of 10.4 MB.
  - Device: 2-layer projection head as bf16 matmuls (PSUM f32 accum),
    L2 normalization via square/ones-matmul-colsum/exp(-0.5 ln(tau x)),
    then per-instance 32x64 [sim_ap | sim_an] similarity blocks packed
    4 instances per 128 PSUM partitions (PE column tiling).  The InfoNCE
    loss skips the max-subtraction (it cancels exactly; |logit| <= 1/tau
    so exp stays finite) and runs per column-group so only the last
    group's chain sits past the final sims.  A mask matmul reduces over
    the 32 anchors; host divides by 32*32, applies the validity mask and
    the loss weight.
  - Column chunks are [512, 512, 224] so the last chunk's serial
    square->colsum->ln->exp->normalize chain is short.
"""

import os
import sys

import numpy as np

if "/opt/trn_rl_repo" not in sys.path:
    sys.path.insert(0, "/opt/trn_rl_repo")

import concourse.bass as bass
import concourse.tile as tile
from concourse import bacc, mybir
from concourse.bass_utils import run_bass_kernel_spmd

try:
    from ml_dtypes import bfloat16 as np_bf16
except ImportError:  # ml_dtypes ships with jax
    import jax.numpy as _jnp

    np_bf16 = _jnp.bfloat16

F32 = mybir.dt.float32
BF16 = mybir.dt.bfloat16

TAU = 0.07
LOSS_WEIGHT = 1.2
NUM_SAMPLES = 32
C = 256
SIDE = 28
PIX = SIDE * SIDE
N_INST = 100
N_CORES = 8
NI = 13                      # instances per core (8*13 = 104 >= 100)
SAMP = 3 * NUM_SAMPLES       # 96 sampled pixels per instance
S = NI * SAMP                # 1248 columns per core
GH = S // 2                  # 624-column DMA halves
CHS = [(0, 512), (512, 512), (1024, 224)]
# contiguous DMA pieces of the flat gtd layout: chunks 0 and 1 are split
# in two 256-col halves so both HWDGE queues fetch them in parallel
PIECES = [(0, 256), (256, 256), (512, 256), (768, 256), (1024, 224)]
NWARM = int(os.environ.get("NWARM", "7"))


def _build_nc():
    nc = bacc.Bacc("TRN2", target_bir_lowering=False)
    gtd = nc.declare_dram_parameter("gtd", [128, 2 * S], BF16, isOutput=False)
    w1d = nc.declare_dram_parameter("w1d", [128, 2, 2, 128], BF16, isOutput=False)
    w2d = nc.declare_dram_parameter("w2d", [128, 2, 2, 128], BF16, isOutput=False)
    auxd = nc.declare_dram_parameter("auxd", [128, 4], F32, isOutput=False)
    lossd = nc.declare_dram_parameter("loss", [128, 4], F32, isOutput=True)

    AT = mybir.ActivationFunctionType
    ALU = mybir.AluOpType
    PSUM = bass.MemorySpace.PSUM
    X = mybir.AxisListType.X

    with tile.TileContext(nc) as tc:
        with tc.tile_pool(name="singles", bufs=1) as sg:
            W1 = sg.tile([128, 2, 2, 128], BF16)
            W2 = sg.tile([128, 2, 2, 128], BF16)
            AUX = sg.tile([128, 4], F32)
            gtc = [
                sg.tile([128, 2, w], BF16, name=f"gt{c}")
                for c, (c0, w) in enumerate(CHS)
            ]
            ones128 = sg.tile([128, 128], BF16)
            wmov = sg.tile([128, 512], BF16)

            nc.vector.memset(ones128[:], 1.0)
            nc.vector.memset(wmov[:], 1.0)

            # One ACT table set covers every function used here (relu,
            # square, ln, exp, copy).  Preload it first on the scalar
            # engine so the framework never auto-inserts a second load.
            nc.scalar.add_instruction(
                mybir.InstLoadActFuncSet(
                    name=nc.get_next_instruction_name(),
                    ins=[],
                    outs=[],
                    act_func_set_id=6,  # natural_log_exp_and_others
                )
            )

            # Input DMAs split across the two HWDGE queues (sync + scalar),
            # pieces aligned to compute chunks, ordered by first use.
            # piece i covers flat elements [2*o, 2*(o+w)) = chunk cols with
            # both k-halves; SBUF side strides into the right gtc tile.
            def gpiece(eng, i):
                o, w = PIECES[i]
                c = 0 if o < 512 else (1 if o < 1024 else 2)
                cbase = CHS[c][0]
                eng.dma_start(
                    out=gtc[c][:, :, o - cbase : o - cbase + w],
                    in_=gtd[:, 2 * o : 2 * (o + w)].rearrange(
                        "p (k j) -> p k j", k=2
                    ),
                )

            gpiece(nc.sync, 0)
            gpiece(nc.scalar, 1)
            nc.sync.dma_start(out=W1[:], in_=w1d[:, :, :, :])
            nc.scalar.dma_start(out=AUX[:], in_=auxd[:, :])
            gpiece(nc.sync, 2)
            gpiece(nc.scalar, 3)
            nc.scalar.dma_start(out=W2[:], in_=w2d[:, :, :, :])
            gpiece(nc.sync, 4)

            BIA = AUX[:, 0:4]

            # PE warm-up during the DMA window: full-width streams so the
            # tensor-engine clock actually ramps before real data lands.
            with tc.tile_pool(name="warmp", bufs=2, space=PSUM) as warmp:
                for _ in range(NWARM):
                    wt = warmp.tile([128, 512], F32, tag="warm")
                    nc.tensor.matmul(
                        wt[:], ones128[:], wmov[:], start=True, stop=True
                    )

            with (
                tc.tile_pool(name="big", bufs=1) as big,
                tc.tile_pool(name="l1p", bufs=2, space=PSUM) as l1p,
                tc.tile_pool(name="l2p", bufs=3, space=PSUM) as l2p,
                tc.tile_pool(name="nsqp", bufs=2, space=PSUM) as nsqp,
                tc.tile_pool(name="simp", bufs=1, space=PSUM) as simp,
            ):
                HS = big.tile([128, 2, S], BF16, name="HS")
                PS = big.tile([128, 2, S], BF16, name="PS")
                QS = big.tile([128, 2, S], BF16, name="QS")
                PN = big.tile([128, 2, S], BF16, name="PN")
                RR = big.tile([128, S], BF16, name="RR")
                spb = simp.tile([128, 256], F32, tag="sp")
                sp = spb[:, 0:256]
                pqs = {}

                def csl(c):
                    c0, w = CHS[c]
                    return slice(c0, c0 + w)

                def l1(c, m, eng):
                    c0, w = CHS[c]
                    pp = l1p.tile([128, 512], F32, tag="pp")
                    for k in (0, 1):
                        nc.tensor.matmul(
                            pp[:, :w], W1[:, m, k], gtc[c][:, k],
                            start=(k == 0), stop=(k == 1),
                        )
                    if eng == "v":
                        nc.vector.tensor_scalar(
                            out=HS[:, m, csl(c)], in0=pp[:, :w],
                            scalar1=BIA[:, m : m + 1], scalar2=0.0,
                            op0=ALU.add, op1=ALU.max,
                        )
                    else:
                        nc.scalar.activation(
                            out=HS[:, m, csl(c)], in_=pp[:, :w], func=AT.Relu,
                            bias=BIA[:, m : m + 1],
                        )

                def l2mm(c, m):
                    c0, w = CHS[c]
                    pq = l2p.tile([128, 512], F32, tag="pq")
                    pqs[(c, m)] = pq
                    for k in (0, 1):
                        nc.tensor.matmul(
                            pq[:, :w], W2[:, m, k], HS[:, k, csl(c)],
                            start=(k == 0), stop=(k == 1),
                        )

                def ps_op(c, m):
                    c0, w = CHS[c]
                    nc.vector.tensor_scalar_add(
                        out=PS[:, m, csl(c)], in0=pqs[(c, m)][:, :w],
                        scalar1=BIA[:, 2 + m : 3 + m],
                    )

                def qs_act(c, m):
                    c0, w = CHS[c]
                    nc.scalar.activation(
                        out=QS[:, m, csl(c)], in_=pqs[(c, m)][:, :w],
                        func=AT.Square, bias=BIA[:, 2 + m : 3 + m],
                    )

                def qs_dve(c, m):
                    nc.vector.tensor_mul(
                        out=QS[:, m, csl(c)], in0=PS[:, m, csl(c)],
                        in1=PS[:, m, csl(c)],
                    )

                def nsqmm(c):
                    c0, w = CHS[c]
                    nq = nsqp.tile([128, 512], F32, tag="nq")
                    for m in (0, 1):
                        nc.tensor.matmul(
                            nq[:, :w], ones128[:], QS[:, m, csl(c)],
                            start=(m == 0), stop=(m == 1),
                        )
                    return nq

                def ln_op(c, nq):
                    c0, w = CHS[c]
                    lnt = big.tile([128, 512], F32, tag="lnt", bufs=2)
                    nc.scalar.activation(
                        out=lnt[:, :w], in_=nq[:, :w], func=AT.Ln, scale=float(TAU)
                    )
                    return lnt

                def exp_op(c, lnt):
                    c0, w = CHS[c]
                    nc.scalar.activation(
                        out=RR[:, csl(c)], in_=lnt[:, :w], func=AT.Exp, scale=-0.5
                    )

                def pn_op(c):
                    c0, w = CHS[c]
                    nc.vector.tensor_mul(
                        out=PN[:, :, csl(c)], in0=PS[:, :, csl(c)],
                        in1=RR[:, csl(c)].unsqueeze(1).broadcast_to([128, 2, w]),
                    )

                def sims(g):
                    for q in range(4):
                        n = 4 * g + q
                        if n >= NI:
                            continue
                        a0 = SAMP * n
                        for k in (0, 1):
                            nc.tensor.matmul(
                                sp[32 * q : 32 * q + 32, 64 * g : 64 * g + 64],
                                PN[:, k, a0 : a0 + 32],
                                PN[:, k, a0 + 32 : a0 + 96],
                                start=(k == 0), stop=(k == 1),
                                tile_position=(0, 32 * q),
                            )

                # pad slots (g=3, q>=1) are never written by a matmul;
                # partition-offset accesses may span at most 32 partitions
                for q in (1, 2, 3):
                    nc.vector.memset(sp[32 * q : 32 * q + 32, 192:256], 0.0)

                # ---- loss tail, two halves of [128, 128] ----
                # loss_km = ln(exp(s_ap) + sum_j exp(s_an_j)) - s_ap
                ee = big.tile([128, 256], F32, name="ee")
                ssum = big.tile([128, 4], F32, name="ssum")
                tt = big.tile([128, 128], F32, name="tt")
                lg = big.tile([128, 128], F32, name="lg")
                ctb = big.tile([128, 128], F32, name="ctb")
                rowr = big.tile([128, 4], F32, name="rowr")

                def tail_exp(h):
                    nc.scalar.activation(
                        out=ee[:, 128 * h : 128 * h + 128],
                        in_=sp[:, 128 * h : 128 * h + 128], func=AT.Exp,
                    )

                def tail_dve1(h):
                    eev = ee[:, 128 * h : 128 * h + 128].rearrange(
                        "p (g x) -> p g x", g=2
                    )
                    nc.vector.reduce_sum(
                        out=ssum[:, 2 * h : 2 * h + 2], in_=eev[:, :, 32:64], axis=X
                    )
                    nc.vector.tensor_add(
                        out=tt[:, 64 * h : 64 * h + 64].rearrange(
                            "p (g x) -> p g x", g=2
                        ),
                        in0=eev[:, :, 0:32],
                        in1=ssum[:, 2 * h : 2 * h + 2]
                        .unsqueeze(-1)
                        .broadcast_to([128, 2, 32]),
                    )

                def tail_ln(h):
                    nc.scalar.activation(
                        out=lg[:, 64 * h : 64 * h + 64],
                        in_=tt[:, 64 * h : 64 * h + 64], func=AT.Ln,
                    )

                def tail_dve2(h):
                    # fused (lg - s_ap) with per-partition row-sum accumulate
                    for g in (2 * h, 2 * h + 1):
                        nc.vector.scalar_tensor_tensor(
                            out=ctb[:, 32 * g : 32 * g + 32],
                            in0=lg[:, 32 * g : 32 * g + 32],
                            scalar=0.0,
                            in1=sp[:, 64 * g : 64 * g + 32],
                            op0=ALU.add,
                            op1=ALU.subtract,
                            accum_out=rowr[:, g : g + 1],
                        )

                # ---- emission (per-engine program order matters) ----
                # PE: warm, L1(0), L1(1), L2(0), L1(2), L2(1), L2(2),
                #     nsq(0..2), sims(0..3), lp  -- all L2 before any nsq
                #     so nsq's qs-waits never idle the PE.
                l1(0, 0, "v"); l1(0, 1, "v")
                l1(1, 0, "s"); l1(1, 1, "s")
                l2mm(0, 0); l2mm(0, 1)
                qs_act(0, 0); ps_op(0, 0); ps_op(0, 1); qs_dve(0, 1)
                l1(2, 0, "v"); l1(2, 1, "s")
                l2mm(1, 0); l2mm(1, 1)
                qs_act(1, 0); ps_op(1, 0); ps_op(1, 1); qs_dve(1, 1)
                l2mm(2, 0); l2mm(2, 1)
                qs_act(2, 0); ps_op(2, 1); qs_dve(2, 1); ps_op(2, 0)
                nq0 = nsqmm(0)
                nq1 = nsqmm(1)
                nq2 = nsqmm(2)
                ln0 = ln_op(0, nq0)
                exp_op(0, ln0)
                ln1 = ln_op(1, nq1)
                exp_op(1, ln1)
                ln2 = ln_op(2, nq2)
                exp_op(2, ln2)
                pn_op(0); pn_op(1); pn_op(2)
                sims(0); sims(1); sims(2); sims(3)
                # stage-major tail emission: each engine's queue never has a
                # later-ready op ahead of an earlier-ready one
                tail_exp(0); tail_exp(1)
                tail_dve1(0); tail_dve1(1)
                tail_ln(0); tail_ln(1)
                tail_dve2(0); tail_dve2(1)

                # host sums each 32-anchor partition block of rowr
                nc.sync.dma_start(out=lossd[:, :], in_=rowr[:])

    nc.compile()
    return nc


_NC_CACHE = None


def _get_nc():
    global _NC_CACHE
    if _NC_CACHE is None:
        _NC_CACHE = _build_nc()
    return _NC_CACHE


def _host_prep(feats, w1, b1, w2, b2, anchor_inds, pos_inds, neg_inds):
    """Build the 8 per-core input maps (host-side gather + packing)."""
    ff = np.asarray(feats, np.float32).reshape(N_INST, C, PIX)

    def flat(i):
        i = np.asarray(i)
        return i[..., 0].astype(np.int64) * SIDE + i[..., 1].astype(np.int64)

    idx = np.concatenate(
        [flat(anchor_inds), flat(pos_inds), flat(neg_inds)], axis=1
    )  # [100, 96]
    ntot = N_CORES * NI
    inst = np.arange(ntot) % N_INST  # wrap the 4 pad rows

    G = np.take_along_axis(ff[inst], idx[inst][:, None, :], axis=2)  # [104,256,96]
    G = G.reshape(N_CORES, NI, C, SAMP).transpose(0, 2, 1, 3)  # [8,256,13,96]
    G = G.reshape(N_CORES, 2, 128, S)  # [core, k, p, col]
    # piece-major, per-piece [k, col] contiguous per partition row
    pieces = [
        G[:, :, :, o : o + w].transpose(0, 2, 1, 3).reshape(N_CORES, 128, 2 * w)
        for (o, w) in PIECES
    ]
    gtd = np.ascontiguousarray(np.concatenate(pieces, axis=2)).astype(np_bf16)

    def wpack(w):
        wa = np.asarray(w, np.float32).reshape(2, 128, 2, 128)  # [m,i,k,p]
        # device layout [p, m, k, i]
        return np.ascontiguousarray(wa.transpose(3, 0, 2, 1)).astype(np_bf16)

    w1d = wpack(w1)
    w2d = wpack(w2)
    b1r = np.asarray(b1, np.float32).reshape(2, 128).T  # [128, 2]
    b2r = np.asarray(b2, np.float32).reshape(2, 128).T
    auxd = np.ascontiguousarray(np.concatenate([b1r, b2r], axis=1))  # [128, 4]

    return [
        {"gtd": gtd[c], "w1d": w1d, "w2d": w2d, "auxd": auxd}
        for c in range(N_CORES)
    ]


def _finalize(loss_per, gt_mask):
    gt = np.asarray(gt_mask)
    area = gt.reshape(gt.shape[0], -1).sum(axis=1)
    valid = (area > NUM_SAMPLES) & (area < PIX - NUM_SAMPLES)
    n_valid = np.float32(valid.sum())
    if n_valid > 0:
        total = np.float32(np.where(valid, loss_per, 0.0).astype(np.float32).sum())
        out = total / max(n_valid, np.float32(1.0))
    else:
        out = np.float32(0.0)
    return np.float32(out * np.float32(LOSS_WEIGHT))


def kernel(feats, w1, b1, w2, b2, gt_mask, anchor_inds, pos_inds, neg_inds,
           _results_hook=None):
    nc = _get_nc()
    in_maps = _host_prep(feats, w1, b1, w2, b2, anchor_inds, pos_inds, neg_inds)
    res = run_bass_kernel_spmd(nc, in_maps, list(range(N_CORES)))
    if _results_hook is not None:
        _results_hook(res)
    loss_per = np.zeros(N_CORES * NI, np.float32)
    for c in range(N_CORES):
        lo = np.asarray(res.results[c]["loss"], np.float32)  # [128, 4]
        blk = lo.reshape(4, 32, 4).sum(axis=1)  # [q, g]
        for n in range(NI):
            loss_per[NI * c + n] = blk[n % 4, n // 4]
    loss_per = loss_per[:N_INST] / float(NUM_SAMPLES * NUM_SAMPLES)
    return _finalize(loss_per, gt_mask)
